# revision 19
# baseline (speedup 1.0000x reference)
"""Cross-attention kernel for TRN2, SPMD over 8 NeuronCores.

Problem: B=8, SQ=4096, SKV=77, D_EMBED=1024, D_CROSS=768, H=16, DH=64.
  q = x @ wq + bq ; k = y @ wk + bk ; v = y @ wv + bv
  out = softmax(q k^T / 8) v @ wo + bo

Sharding: pure data-parallel over batch (1 batch element per core, no
collectives). Host pre-transposes x and y per core; every device tensor is
feature-major (contraction dim on partitions) until the O-projection, which
uses attnout^T as the stationary operand so the output lands row-major.

Perf notes (~383us, vs 629us for the unpipelined version of the same math):
  - Depth-3 software pipeline over 512-query chunks: iteration `it` issues
    attention passes for chunk it-1, Q-proj for chunk it, and O-proj for
    chunk it-2.  Dense projection matmuls fill the PE bubbles left by the
    scalar-engine exp chain, keeping the PE free of idle windows >3.4us so
    the HAM clock gate stays at 8/8 (2.4 GHz); without this the clock
    oscillates at 4/8 and the kernel runs ~2x slower.  The trailing O-proj
    makes the kernel tail a dense matmul stream.
  - ~14 dummy matmuls at the head span the input-DMA wait so the PE is
    already warm when real work starts.
  - No bias matmuls on the PE: q bias is a per-partition activation bias,
    o bias is a host-prebroadcast [128,1024] tile added by the DVE during
    the PSUM->SBUF evacuation.
  - Scores for a head pair run as two concurrent 64-row matmuls (row-group
    packing via base_partition 0/64) into one 2-bank PSUM tile; one exp
    activation covers the pair.
  - Softmax without max-subtraction (scores are O(8) here; exp fits fp32).
    Normalization: sums via sel16 gather matmuls into [16,512],
    reciprocal_approx_fast on DVE, broadcast back to 128 partitions via
    two concurrent rank-1 matmuls (col packing), applied by the DVE while
    evacuating PV PSUM.
  - PSUM is the scarce resource (8 banks):  mm 2 + scores 2 + sums 1 +
    rb 1 + pv 1 + o 1.  The epilogue O-proj rotates its accumulator
    through the slots freed by the wound-down attention phases.
  - Host-side prep (free): per-chunk pre-rearranged x, pre-transposed y,
    per-column-tile wq/wk slices so every DMA is one contiguous
    descriptor per partition and the first compute gates on ~1.6MB.

Compute dtype: bf16 operands (host-cast), fp32 PSUM accumulation, fp32 out.
"""

import numpy as np
import ml_dtypes

import concourse.bass as bass
import concourse.mybir as mybir
import concourse.tile as tile
from concourse import bacc
from concourse import bass_utils

F32 = mybir.dt.float32
BF16 = mybir.dt.bfloat16
AF = mybir.ActivationFunctionType

B = 8
SQ = 4096
SKV = 77
D = 1024
DC = 768
H = 16
DH = 64
KT = D // 128    # 8 embed k-tiles
KC = DC // 128   # 6 cross k-tiles
CT = D // 128    # 8 column tiles of the 1024-wide projections
CH = 512         # query chunk
NCH = SQ // CH   # 8 chunks
NQT = CH // 128  # 4 query 128-tiles per chunk
HP = H // 2      # 8 head pairs

_CACHED = {}


def _build():
    nc = bacc.Bacc("TRN2", target_bir_lowering=False, debug=False, num_devices=B)

    # host pre-rearranged: xt[c] is chunk c in [128, KT, CH] layout,
    # yt is [128, KC, SKV] — one contiguous descriptor per partition
    xt = nc.dram_tensor("xt", (NCH, 128, KT * CH), BF16, kind="ExternalInput")
    yt = nc.dram_tensor("yt", (128, KC * SKV), BF16, kind="ExternalInput")
    # per-column-tile slices, host-prerearranged: [ct][128p][kt*128]
    wq_d = nc.dram_tensor("wq", (CT, 128, KT * 128), BF16, kind="ExternalInput")
    wk_d = nc.dram_tensor("wk", (CT, 128, KC * 128), BF16, kind="ExternalInput")
    wv_d = nc.dram_tensor("wv", (DC, D), BF16, kind="ExternalInput")
    wo_d = nc.dram_tensor("wo", (D, D), BF16, kind="ExternalInput")
    bqp_d = nc.dram_tensor("bqp", (128, CT), F32, kind="ExternalInput")
    bk8p_d = nc.dram_tensor("bk8p", (128, CT), F32, kind="ExternalInput")
    bv_d = nc.dram_tensor("bv", (1, D), BF16, kind="ExternalInput")
    bo128_d = nc.dram_tensor("bo128", (128, D), F32, kind="ExternalInput")
    sel16_d = nc.dram_tensor("sel16", (SKV, H * 16), BF16, kind="ExternalInput")
    sel64_d = nc.dram_tensor("sel64", (16, H * DH), BF16, kind="ExternalInput")
    out_d = nc.dram_tensor("out", (SQ, D), F32, kind="ExternalOutput")

    with tile.TileContext(nc) as tc:
        with (
            tc.tile_pool(name="consts", bufs=1) as consts,
            tc.tile_pool(name="wpool", bufs=1) as wpool,
            tc.tile_pool(name="xpool", bufs=3) as xpool,
            tc.tile_pool(name="qpool", bufs=3) as qpool,
            tc.tile_pool(name="epool", bufs=12) as epool,
            tc.tile_pool(name="rp", bufs=2) as rp,
            tc.tile_pool(name="rbp", bufs=10) as rbp,
            tc.tile_pool(name="aopool", bufs=3) as aopool,
            tc.tile_pool(name="opool", bufs=3) as opool,
            tc.tile_pool(name="pmm", bufs=2, space="PSUM") as pmm,
            tc.tile_pool(name="psc", bufs=1, space="PSUM") as psc,
            tc.tile_pool(name="prb", bufs=1, space="PSUM") as prb,
            tc.tile_pool(name="ppv", bufs=1, space="PSUM") as ppv,
            tc.tile_pool(name="pnrm", bufs=1, space="PSUM") as pnrm,
            tc.tile_pool(name="pso", bufs=1, space="PSUM") as psop,
        ):
            # ---- weights / constants ----
            # DMA order matters for the pipeline head: the small k/v-proj
            # inputs land first so the PE starts within ~5us, then x chunk 0
            # and wq for the first q-projection, then everything else.
            yt_sb = consts.tile([128, KC, SKV], BF16, tag="yt")
            nc.sync.dma_start(yt_sb[:], yt.ap())
            # layout [128, ct, kt, 128]: per-ct slices land independently so
            # the first k-proj group starts after slice 0 (not the full 1.5MB)
            wk_sb = wpool.tile([128, CT, KC, 128], BF16, tag="wk")
            for ct in range(CT):
                nc.sync.dma_start(wk_sb[:, ct], wk_d.ap()[ct])
            bk8p_sb = consts.tile([128, CT], F32, tag="bk8p")
            nc.sync.dma_start(bk8p_sb[:], bk8p_d.ap())
            ones77r = consts.tile([1, SKV], BF16, tag="ones77r")
            nc.vector.memset(ones77r[:], 1.0)

            # wq slice 0 lands before xT0: the first q-proj group only needs
            # slice 0, so its DMA gate shrinks by the other 1.75MB of wq
            wq_sb = wpool.tile([128, CT, KT, 128], BF16, tag="wq")
            nc.sync.dma_start(wq_sb[:, 0], wq_d.ap()[0])
            xT_tiles = {}
            xT_tiles[0] = xpool.tile([128, KT, CH], BF16, tag="xT", name="xT")
            nc.sync.dma_start(xT_tiles[0][:], xt.ap()[0])
            for ct in range(1, CT):
                nc.sync.dma_start(wq_sb[:, ct], wq_d.ap()[ct])
            bqp_sb = consts.tile([128, CT], F32, tag="bqp")
            nc.sync.dma_start(bqp_sb[:], bqp_d.ap())

            wv_sb = wpool.tile([128, KC, D], BF16, tag="wv")
            nc.sync.dma_start(wv_sb[:], wv_d.ap().rearrange("(kt p) n -> p kt n", p=128))
            bv_sb = consts.tile([1, D], BF16, tag="bv")
            nc.sync.dma_start(bv_sb[:], bv_d.ap())

            wo_sb = wpool.tile([128, KT, D], BF16, tag="wo")
            nc.sync.dma_start(wo_sb[:], wo_d.ap().rearrange("(kt p) n -> p kt n", p=128))
            bo128_sb = consts.tile([128, D], F32, tag="bo128")
            nc.sync.dma_start(bo128_sb[:], bo128_d.ap())
            sel16_sb = consts.tile([SKV, H * 16], BF16, tag="sel16")
            nc.sync.dma_start(sel16_sb[:], sel16_d.ap())
            sel64_sb = consts.tile([16, H * DH], BF16, tag="sel64")
            nc.sync.dma_start(sel64_sb[:], sel64_d.ap())

            kT_sb = consts.tile([128, CT, SKV], BF16, tag="kT")
            v_sb = consts.tile([SKV, H, DH], BF16, tag="v")

            # ---- PE warm-up: dummy matmuls spanning the input-DMA wait so the
            # HAM clock gate reaches 8/8 before real work and stays there
            warm_in = consts.tile([128, 512], BF16, tag="warm")
            nc.vector.memset(warm_in[:], 0.0)
            # 14 N=512 matmuls span the ~7.5-13.5us window until the k-proj
            # DMAs land; ending earlier would let the HAM MID window (~3.4us
            # of PE idle) re-throttle the clock before real work begins
            ps_warm = psop.tile([128, 512], F32, tag="o", name="ps_warm")
            for i in range(14):
                nc.tensor.matmul(
                    ps_warm[:], warm_in[:, 0:128], warm_in[:],
                    start=(i == 0), stop=(i == 13),
                )

            # ---- k projection: kT[c, s] = sum_k wk[k, c] yT[k, s]; fold (.+bk)/8 ----
            for ct in range(CT):
                psk = pmm.tile([128, CH], F32, tag="mm")
                for kt in range(KC):
                    nc.tensor.matmul(
                        psk[:, 0:SKV],
                        wk_sb[:, ct, kt, :],
                        yt_sb[:, kt, :],
                        start=(kt == 0),
                        stop=(kt == KC - 1),
                    )
                nc.scalar.activation(
                    kT_sb[:, ct, :],
                    psk[:, 0:SKV],
                    AF.Identity,
                    scale=0.125,
                    bias=bk8p_sb[:, ct:ct + 1],
                )

            qT_tiles = {}
            aoT_tiles = {}

            # ---- software-pipelined main loop (depth 3) ----
            # iteration it: attnA+attnB for chunk it-1, q-proj for chunk it,
            # o-proj for chunk it-2 (trails so the kernel tail is a dense
            # matmul stream instead of a dependency chain).
            for it in range(NCH + 2):
                # prefetch xT for chunk it+1
                if it + 1 < NCH:
                    xT_tiles[it + 1] = xpool.tile([128, KT, CH], BF16, tag="xT", name="xT")
                    nc.sync.dma_start(xT_tiles[it + 1][:], xt.ap()[it + 1])

                # ---- attention pass A for chunk it-1 ----
                e_pairs = {}
                if 1 <= it <= NCH:
                    qT_prev = qT_tiles.pop(it - 1)
                    ps_sum = pnrm.tile([16, CH], F32, tag="nrm")
                    for hp in range(HP):
                        pssc = psc.tile([SKV, 2, CH], F32, tag="sc")
                        for i in range(2):
                            h = 2 * hp + i
                            nc.tensor.matmul(
                                pssc[:, i, :],
                                kT_sb[i * 64:(i + 1) * 64, hp, :],
                                qT_prev[i * 64:(i + 1) * 64, hp, :],
                                start=True, stop=True,
                            )
                        e_pair = epool.tile([SKV, 2, CH], BF16, tag="e")
                        nc.scalar.activation(e_pair[:], pssc[:], AF.Exp)
                        e_pairs[hp] = e_pair
                        for i in range(2):
                            h = 2 * hp + i
                            nc.tensor.matmul(
                                ps_sum[:],
                                sel16_sb[:, h * 16:(h + 1) * 16],
                                e_pair[:, i, :],
                                start=(h == 0), stop=(h == H - 1),
                                skip_group_check=True,
                            )
                    r16f = rp.tile([16, CH], F32, tag="rf", name="r16f")
                    nc.vector.reciprocal_approx_fast(r16f[:], ps_sum[:])
                    r16 = rp.tile([16, CH], BF16, tag="r", name="r16")
                    nc.scalar.copy(r16[:], r16f[:])

                # ---- q projection for chunk it ----
                if it < NCH:
                    xT_ch = xT_tiles.pop(it)
                    qT = qpool.tile([128, CT, CH], BF16, tag="qT")
                    qT_tiles[it] = qT
                    for ct in range(CT):
                        psq = pmm.tile([128, CH], F32, tag="mm")
                        for kt in range(KT):
                            nc.tensor.matmul(
                                psq[:],
                                wq_sb[:, ct, kt, :],
                                xT_ch[:, kt, :],
                                start=(kt == 0),
                                stop=(kt == KT - 1),
                            )
                        nc.vector.tensor_scalar_add(
                            qT[:, ct, :], psq[:], bqp_sb[:, ct:ct + 1],
                        )

                # ---- v projection (once, after q-proj(0): wv lands during
                # q-proj compute, so the PE never waits on it) ----
                if it == 0:
                    for n in range(2):
                        # psop pool, not pmm: sharing the pmm slot rotation
                        # would chain q-proj(0) behind the wv DMA
                        psv = psop.tile([128, CH], F32, tag="o", name="psv")
                        for kt in range(KC):
                            nc.tensor.matmul(
                                psv[0:SKV, :],
                                yt_sb[:, kt, :],
                                wv_sb[:, kt, n * 512:(n + 1) * 512],
                                start=(kt == 0),
                                stop=False,
                            )
                        nc.tensor.matmul(
                            psv[0:SKV, :],
                            ones77r[:],
                            bv_sb[0:1, n * 512:(n + 1) * 512],
                            start=False,
                            stop=True,
                        )
                        nc.vector.tensor_copy(
                            v_sb[:, n * 8:(n + 1) * 8, :], psv[0:SKV, :]
                        )

                # ---- attention pass B for chunk it-1 ----
                if 1 <= it <= NCH:
                    # reciprocal-broadcast block: depends only on r16, so the
                    # rank-1 matmuls + DVE copies run ahead of the PV chain
                    rb_sbs = {}
                    for hp in range(HP):
                        rb_ps = prb.tile([128, CH], F32, tag="rb")
                        for i in range(2):
                            h = 2 * hp + i
                            nc.tensor.matmul(
                                rb_ps[i * 64:(i + 1) * 64, :],
                                sel64_sb[:, h * 64:(h + 1) * 64],
                                r16[:],
                                start=True, stop=True,
                            )
                        rb_sb = rbp.tile([128, CH], BF16, tag="rbsb", name="rb_sb")
                        nc.scalar.copy(rb_sb[:], rb_ps[:])
                        rb_sbs[hp] = rb_sb

                    aoT = aopool.tile([128, KT, CH], BF16, tag="aoT")
                    aoT_tiles[it - 1] = aoT
                    for hp in range(HP):
                        pspv = ppv.tile([128, CH], F32, tag="pv")
                        for i in range(2):
                            h = 2 * hp + i
                            nc.tensor.matmul(
                                pspv[i * 64:(i + 1) * 64, :],
                                v_sb[:, h, :],
                                e_pairs[hp][:, i, :],
                                start=True, stop=True,
                            )
                        nc.vector.tensor_mul(aoT[:, hp, :], pspv[:], rb_sbs[hp][:])

                # ---- o-projection for chunk it-2 ----
                if it >= 2:
                    aoT_prev = aoT_tiles.pop(it - 2)
                    q0 = (it - 2) * CH
                    for qt in range(NQT):
                        for n in range(2):
                            g = qt * 2 + n
                            # epilogue iterations have no q-proj filler left;
                            # rotate the accumulator through PSUM slots freed
                            # by the wound-down attention phases
                            if it == NCH + 1:
                                pool, tg = [(psop, "o"), (psc, "sc"),
                                            (ppv, "pv")][g % 3]
                                pso = pool.tile([128, 512], F32, tag=tg,
                                                name="pso_e")
                            elif it == NCH and g % 3 == 1:
                                pso = ppv.tile([128, 512], F32, tag="pv",
                                               name="pso_pv")
                            elif it == NCH and g % 3 == 2:
                                pso = prb.tile([128, 512], F32, tag="rb",
                                               name="pso_rb")
                            else:
                                pso = psop.tile([128, 512], F32, tag="o")
                            for kt in range(KT):
                                nc.tensor.matmul(
                                    pso[:],
                                    aoT_prev[:, kt, qt * 128:(qt + 1) * 128],
                                    wo_sb[:, kt, n * 512:(n + 1) * 512],
                                    start=(kt == 0),
                                    stop=(kt == KT - 1),
                                )
                            o_sb = opool.tile([128, 512], F32, tag="osb")
                            nc.vector.tensor_add(
                                o_sb[:], pso[:], bo128_sb[:, n * 512:(n + 1) * 512]
                            )
                            nc.sync.dma_start(
                                out_d.ap()[q0 + qt * 128: q0 + (qt + 1) * 128,
                                           n * 512:(n + 1) * 512],
                                o_sb[:],
                            )

    nc.compile()
    return nc


def _get_nc():
    if "nc" not in _CACHED:
        _CACHED["nc"] = _build()
    return _CACHED["nc"]


def make_in_maps(inputs):
    x = np.asarray(inputs["x"])
    y = np.asarray(inputs["y"])
    bf = ml_dtypes.bfloat16
    # (K, D) -> (CT, 128, KTx*128): slice ct holds columns ct*128..(ct+1)*128
    # transposed into [partition=col, kt, 128] contiguous per partition
    def wslices(w, ktn):
        w = np.asarray(w).astype(bf)          # (K, D)
        w = w.reshape(ktn, 128, CT, 128)      # (kt, p, ct, n)
        return np.ascontiguousarray(w.transpose(2, 1, 0, 3)).reshape(
            CT, 128, ktn * 128)
    wq_b = wslices(inputs["wq"], KT)
    wk_b = wslices(inputs["wk"], KC)
    wv_b = np.asarray(inputs["wv"]).astype(bf)
    wo_b = np.asarray(inputs["wo"]).astype(bf)
    bqp = np.ascontiguousarray(
        np.asarray(inputs["bq"]).reshape(CT, 128).T).astype(np.float32)
    bk8p = np.ascontiguousarray(
        (np.asarray(inputs["bk"]).reshape(CT, 128).T * 0.125)).astype(np.float32)
    bv_b = np.asarray(inputs["bv"]).reshape(1, D).astype(bf)
    bo128 = np.broadcast_to(
        np.asarray(inputs["bo"]).reshape(1, D).astype(np.float32), (128, D)
    ).copy()
    sel16 = np.zeros((SKV, H, 16), np.float32)
    sel16[:, np.arange(H), np.arange(16)] = 1.0
    sel16 = sel16.reshape(SKV, H * 16).astype(bf)
    sel64 = np.zeros((16, H, DH), np.float32)
    sel64[np.arange(16), np.arange(H), :] = 1.0
    sel64 = sel64.reshape(16, H * DH).astype(bf)

    in_maps = []
    for b in range(B):
        # xt: (D, SQ) -> chunks (NCH, 128, KT*CH); yt: (DC, SKV) -> (128, KC*SKV)
        xtb = x[b].T.astype(bf).reshape(KT, 128, NCH, CH)
        xtb = np.ascontiguousarray(xtb.transpose(2, 1, 0, 3)).reshape(
            NCH, 128, KT * CH)
        ytb = y[b].T.astype(bf).reshape(KC, 128, SKV)
        ytb = np.ascontiguousarray(ytb.transpose(1, 0, 2)).reshape(128, KC * SKV)
        in_maps.append({
            "xt": xtb,
            "yt": ytb,
            "wq": wq_b, "wk": wk_b, "wv": wv_b, "wo": wo_b,
            "bqp": bqp, "bk8p": bk8p, "bv": bv_b, "bo128": bo128,
            "sel16": sel16, "sel64": sel64,
        })
    return in_maps


def kernel(x, y, wq, bq, wk, bk, wv, bv, wo, bo):
    in_maps = make_in_maps(dict(
        x=x, y=y, wq=wq, bq=bq, wk=wk, bk=bk, wv=wv, bv=bv, wo=wo, bo=bo))
    nc = _get_nc()
    res = bass_utils.run_bass_kernel_spmd(nc, in_maps, core_ids=list(range(B)))
    out = np.stack([res.results[b]["out"] for b in range(B)], axis=0)
    return out.astype(np.float32)
